# revision 6
# baseline (speedup 1.0000x reference)
"""DistMult decoder kernel for 8 Trainium2 NeuronCores.

Computes out = (input1 * weight[type_index]) @ input2.T + bias with
input1 [8192, 512], input2 [8192, 512] in fp32, out [8192, 8192].

Sharding: rows of input1 (and thus rows of the output) are split across
the 8 cores; input2 / weight / bias are replicated. No communication.

Per-core device program (M = 1024 rows):
  - lhsT  [512, 1024]  = w_r-scaled shard of input1, transposed + cast
    to fp16 on host (K-major); rhs [512, 8192] = input2 transposed +
    cast to fp16 on host.  fp16 runs the PE at 1 cycle/row with fp32
    PSUM accumulation; the whole rhs (64 KB/partition) + lhsT are SBUF
    resident so there is no mid-stream prefetch.
  - compute iterates 16 column slices of 512; all 8 m-tiles accumulate
    a slice in 8 PSUM banks, so one 512 KB rhs slice unlocks ~6.9 us
    of PE work and DMA arrival order matches compute order.
  - every dma_start costs ~600 ns on its sequencer and each DGE ring
    sustains ~100-170 GB/s, so the head uses FEW, LARGE loads spread
    over all three rings in exact consumption order: sync carries
    bias + slice0, scalar carries lhsT[0:256] + slice1, gpsimd carries
    the rest of lhsT and then slices 2-15.  The PE warmup bridges the
    ~11 us until slice0 lands and the stream then runs gapless.
  - output is stored as fp16 (16 MB/core instead of 32 MB) and upcast
    to fp32 on the host.  Stores alternate sync/scalar, and from
    slice 10 the (by then idle) gpsimd ring joins the rotation so the
    store stream never backlogs into a tail drain.
  - PSUM -> SBUF copy + bias add alternates between ACT and DVE.
"""

import os

import numpy as np

import concourse.bacc as bacc
import concourse.mybir as mybir
from concourse.bass_utils import run_bass_kernel_spmd
from concourse.tile import TileContext

N_CORES = 8
N1, N2, D = 8192, 8192, 512
M = N1 // N_CORES  # rows per core
P = 128            # partitions
KT = D // P        # 4 k-tiles
MT = M // P        # 8 m-tiles
NFREE = 512        # psum bank free size (fp32)
NSL = N2 // NFREE  # 16 column slices

# test.py hooks: set TRACE=True before calling kernel() to profile; the
# BassKernelResults of the last run lands in LAST_RESULTS.
TRACE = os.environ.get("BASS_KERNEL_TRACE", "0") == "1"
LAST_RESULTS = None

_cached_nc = None


def _build():
    nc = bacc.Bacc(
        "TRN2", target_bir_lowering=False, debug=False, enable_asserts=False, num_devices=N_CORES
    )
    f32 = mybir.dt.float32
    f16 = mybir.dt.float16
    lhsT = nc.dram_tensor("lhsT", [D, M], f16, kind="ExternalInput")
    rhs = nc.dram_tensor("rhs", [D, N2], f16, kind="ExternalInput")
    biasv = nc.dram_tensor("biasv", [P, 1], f32, kind="ExternalInput")
    out = nc.dram_tensor("out", [M, N2], f16, kind="ExternalOutput")

    # K-major DRAM views split into [P, KT, cols] for single-DMA loads.
    lhsT_r = lhsT[:, :].rearrange("(kt p) m -> p kt m", p=P)
    rhs_r = rhs[:, :].rearrange("(kt p) n -> p kt n", p=P)

    with TileContext(nc) as tc:
        with (
            tc.tile_pool(name="const", bufs=1) as constp,
            tc.tile_pool(name="lhs", bufs=1) as lhsp,
            tc.tile_pool(name="rhsp", bufs=1) as rhsp,
            tc.tile_pool(name="outp", bufs=12) as outp,
            tc.tile_pool(name="psum", bufs=8, space="PSUM") as psump,
        ):
            lt = lhsp.tile([P, KT, M], f16, tag="lhs")
            rt = rhsp.tile([P, KT, N2], f16, tag="rhs")
            bias_t = constp.tile([P, 1], f32, tag="bias")

            def rt_sl(s):
                return rt[:, :, s * NFREE : (s + 1) * NFREE]

            def rhs_sl(s):
                return rhs_r[:, :, s * NFREE : (s + 1) * NFREE]

            # Head: few, large loads in consumption order on all three
            # rings (see module docstring).
            nc.sync.dma_start(out=bias_t[:], in_=biasv[:, :])
            nc.sync.dma_start(out=rt[:, 0:2, 0:NFREE], in_=rhs_r[:, 0:2, 0:NFREE])
            nc.sync.dma_start(out=rt[:, 2:4, 0:NFREE], in_=rhs_r[:, 2:4, 0:NFREE])
            nc.scalar.dma_start(out=lt[:, :, 0:256], in_=lhsT_r[:, :, 0:256])
            nc.scalar.dma_start(
                out=rt[:, 0:2, NFREE : 2 * NFREE],
                in_=rhs_r[:, 0:2, NFREE : 2 * NFREE],
            )
            nc.scalar.dma_start(
                out=rt[:, 2:4, NFREE : 2 * NFREE],
                in_=rhs_r[:, 2:4, NFREE : 2 * NFREE],
            )
            for c0, c1 in ((256, 512), (512, 768), (768, M)):
                nc.gpsimd.dma_start(out=lt[:, :, c0:c1], in_=lhsT_r[:, :, c0:c1])
            for s in range(2, NSL):
                nc.gpsimd.dma_start(out=rt_sl(s), in_=rhs_sl(s))

            # Warm up the PE's HAM clock gate during the head-load
            # window: dummy matmuls push the PE through its ~3.4 us ramp
            # to 2.4 GHz while the loads land.
            warm_w = constp.tile([P, P], f16, tag="warmw")
            warm_r = constp.tile([P, NFREE], f16, tag="warmr")
            nc.vector.memset(warm_w[:], 0.0)
            nc.vector.memset(warm_r[:], 0.0)
            wps = psump.tile([P, NFREE], f32, tag="ps")
            NWARM = 10
            for i in range(NWARM):
                nc.tensor.matmul(
                    wps[:], warm_w[:], warm_r[:],
                    start=(i == 0), stop=(i == NWARM - 1),
                )

            for s in range(NSL):
                cols = slice(s * NFREE, (s + 1) * NFREE)
                for m in range(MT):
                    ps = psump.tile([P, NFREE], f32, tag="ps")
                    for k in range(KT):
                        nc.tensor.matmul(
                            ps[:], lt[:, k, m * P : (m + 1) * P],
                            rt[:, k, cols],
                            start=(k == 0), stop=(k == KT - 1),
                        )
                    ot = outp.tile([P, NFREE], f16, tag="ot")
                    # Alternate psum->sbuf+bias between ACT and the DVE so
                    # neither engine serializes the psum pool.
                    if m % 2 == 0:
                        nc.scalar.activation(
                            ot[:], ps[:],
                            mybir.ActivationFunctionType.Identity,
                            bias=bias_t[:, 0:1],
                        )
                    else:
                        nc.vector.tensor_scalar_add(ot[:], ps[:], bias_t[:, 0:1])
                    if s < 10:
                        st = nc.sync if m % 2 == 0 else nc.scalar
                    else:
                        st = (nc.sync, nc.scalar, nc.gpsimd)[(s * MT + m) % 3]
                    st.dma_start(out=out[m * P : (m + 1) * P, cols], in_=ot[:])
    nc.compile()
    return nc


def kernel(input1, input2, weight, bias, type_index):
    global _cached_nc, LAST_RESULTS

    input1 = np.asarray(input1, dtype=np.float32)
    input2 = np.asarray(input2, dtype=np.float32)
    weight = np.asarray(weight, dtype=np.float32)
    bias = np.asarray(bias, dtype=np.float32).reshape(-1)
    w_r = weight[int(type_index)]  # [D]

    # Host-side prep: fold the w_r row-scale into input1, lay both GEMM
    # operands out K-major, cast to fp16 (device accumulates in fp32).
    scaled = input1 * w_r[None, :]  # [N1, D]
    rhsT = np.ascontiguousarray(input2.T.astype(np.float16))  # [D, N2]
    bias_vec = np.full((P, 1), float(bias[0]), dtype=np.float32)

    in_maps = []
    for c in range(N_CORES):
        shard = scaled[c * M : (c + 1) * M]  # [M, D]
        in_maps.append(
            {
                "lhsT": np.ascontiguousarray(shard.T.astype(np.float16)),
                "rhs": rhsT,
                "biasv": bias_vec,
            }
        )

    if _cached_nc is None:
        _cached_nc = _build()

    res = run_bass_kernel_spmd(
        _cached_nc, in_maps, core_ids=list(range(N_CORES)), trace=TRACE
    )
    LAST_RESULTS = res
    return np.concatenate(
        [res.results[c]["out"] for c in range(N_CORES)], axis=0
    ).astype(np.float32)


# revision 7
# speedup vs baseline: 1.1926x; 1.1926x over previous
"""DistMult decoder kernel for 8 Trainium2 NeuronCores.

Computes out = (input1 * weight[type_index]) @ input2.T + bias with
input1 [8192, 512], input2 [8192, 512] in fp32, out [8192, 8192].

Sharding: rows of input1 (and thus rows of the output) are split across
the 8 cores; input2 / weight / bias are replicated. No communication.

Per-core device program (M = 1024 rows):
  - lhsT  [512, 1024]  = w_r-scaled shard of input1, transposed + cast
    to fp16 on host (K-major); rhs [512, 8192] = input2 transposed +
    cast to fp16 on host.  fp16 runs the PE at 1 cycle/row with fp32
    PSUM accumulation; the whole rhs (64 KB/partition) + lhsT are SBUF
    resident so there is no mid-stream prefetch.
  - compute iterates 16 column slices of 512; all 8 m-tiles accumulate
    a slice in 8 PSUM banks, so one 512 KB rhs slice unlocks ~6.9 us
    of PE work and DMA arrival order matches compute order.
  - every dma_start costs ~600 ns on its sequencer and each DGE ring
    sustains ~100-170 GB/s, so the head uses FEW, LARGE loads spread
    over all three rings in exact consumption order: sync carries
    bias + slice0, scalar carries lhsT[0:256] + slice1, gpsimd carries
    the rest of lhsT and then slices 2-15.  The PE warmup bridges the
    ~11 us until slice0 lands and the stream then runs gapless.
  - output is stored as fp16 (16 MB/core instead of 32 MB) and upcast
    to fp32 on the host.  Stores alternate sync/scalar, and from
    slice 10 the (by then idle) gpsimd ring joins the rotation so the
    store stream never backlogs into a tail drain.
  - PSUM -> SBUF copy + bias add alternates between ACT and DVE.
"""

import os

import numpy as np

import concourse.bacc as bacc
import concourse.mybir as mybir
from concourse.bass_utils import run_bass_kernel_spmd
from concourse.tile import TileContext

N_CORES = 8
N1, N2, D = 8192, 8192, 512
M = N1 // N_CORES  # rows per core
P = 128            # partitions
KT = D // P        # 4 k-tiles
MT = M // P        # 8 m-tiles
NFREE = 512        # psum bank free size (fp32)
NSL = N2 // NFREE  # 16 column slices

# test.py hooks: set TRACE=True before calling kernel() to profile; the
# BassKernelResults of the last run lands in LAST_RESULTS.
TRACE = os.environ.get("BASS_KERNEL_TRACE", "0") == "1"
LAST_RESULTS = None

_cached_nc = None


def _build():
    nc = bacc.Bacc(
        "TRN2", target_bir_lowering=False, debug=False, enable_asserts=False, num_devices=N_CORES
    )
    f32 = mybir.dt.float32
    f16 = mybir.dt.float16
    lhsT = nc.dram_tensor("lhsT", [D, M], f16, kind="ExternalInput")
    rhs = nc.dram_tensor("rhs", [D, N2], f16, kind="ExternalInput")
    biasv = nc.dram_tensor("biasv", [P, 1], f32, kind="ExternalInput")
    out = nc.dram_tensor("out", [M, N2], f16, kind="ExternalOutput")

    # K-major DRAM views split into [P, KT, cols] for single-DMA loads.
    lhsT_r = lhsT[:, :].rearrange("(kt p) m -> p kt m", p=P)
    rhs_r = rhs[:, :].rearrange("(kt p) n -> p kt n", p=P)

    with TileContext(nc) as tc:
        with (
            tc.tile_pool(name="const", bufs=1) as constp,
            tc.tile_pool(name="lhs", bufs=1) as lhsp,
            tc.tile_pool(name="rhsp", bufs=1) as rhsp,
            tc.tile_pool(name="outp", bufs=12) as outp,
            tc.tile_pool(name="psum", bufs=8, space="PSUM") as psump,
        ):
            lt = lhsp.tile([P, KT, M], f16, tag="lhs")
            rt = rhsp.tile([P, KT, N2], f16, tag="rhs")
            bias_t = constp.tile([P, 1], f32, tag="bias")

            def rt_sl(s):
                return rt[:, :, s * NFREE : (s + 1) * NFREE]

            def rhs_sl(s):
                return rhs_r[:, :, s * NFREE : (s + 1) * NFREE]

            # Head: few, large loads in consumption order on all three
            # rings (see module docstring).
            nc.sync.dma_start(out=bias_t[:], in_=biasv[:, :])
            nc.sync.dma_start(out=rt[:, 0:2, 0:NFREE], in_=rhs_r[:, 0:2, 0:NFREE])
            nc.sync.dma_start(out=rt[:, 2:4, 0:NFREE], in_=rhs_r[:, 2:4, 0:NFREE])
            nc.scalar.dma_start(out=lt[:, :, 0:256], in_=lhsT_r[:, :, 0:256])
            nc.scalar.dma_start(
                out=rt[:, 0:2, NFREE : 2 * NFREE],
                in_=rhs_r[:, 0:2, NFREE : 2 * NFREE],
            )
            nc.scalar.dma_start(
                out=rt[:, 2:4, NFREE : 2 * NFREE],
                in_=rhs_r[:, 2:4, NFREE : 2 * NFREE],
            )
            for c0, c1 in ((256, 512), (512, 768), (768, M)):
                nc.gpsimd.dma_start(out=lt[:, :, c0:c1], in_=lhsT_r[:, :, c0:c1])
            for s in range(2, NSL):
                nc.gpsimd.dma_start(out=rt_sl(s), in_=rhs_sl(s))

            # Warm up the PE's HAM clock gate during the head-load
            # window: dummy matmuls push the PE through its ~3.4 us ramp
            # to 2.4 GHz while the loads land.
            warm_w = constp.tile([P, P], f16, tag="warmw")
            warm_r = constp.tile([P, NFREE], f16, tag="warmr")
            nc.vector.memset(warm_w[:], 0.0)
            nc.vector.memset(warm_r[:], 0.0)
            wps = psump.tile([P, NFREE], f32, tag="ps")
            NWARM = 10
            for i in range(NWARM):
                nc.tensor.matmul(
                    wps[:], warm_w[:], warm_r[:],
                    start=(i == 0), stop=(i == NWARM - 1),
                )

            for s in range(NSL):
                cols = slice(s * NFREE, (s + 1) * NFREE)
                for m in range(MT):
                    ps = psump.tile([P, NFREE], f32, tag="ps")
                    for k in range(KT):
                        nc.tensor.matmul(
                            ps[:], lt[:, k, m * P : (m + 1) * P],
                            rt[:, k, cols],
                            start=(k == 0), stop=(k == KT - 1),
                        )
                    ot = outp.tile([P, NFREE], f16, tag="ot")
                    # Alternate psum->sbuf+bias between ACT and the DVE so
                    # neither engine serializes the psum pool.
                    if m % 2 == 0:
                        nc.scalar.activation(
                            ot[:], ps[:],
                            mybir.ActivationFunctionType.Identity,
                            bias=bias_t[:, 0:1],
                        )
                    else:
                        nc.vector.tensor_scalar_add(ot[:], ps[:], bias_t[:, 0:1])
                    st = nc.sync if m % 2 == 0 else nc.scalar
                    st.dma_start(out=out[m * P : (m + 1) * P, cols], in_=ot[:])
    nc.compile()
    return nc


def kernel(input1, input2, weight, bias, type_index):
    global _cached_nc, LAST_RESULTS

    input1 = np.asarray(input1, dtype=np.float32)
    input2 = np.asarray(input2, dtype=np.float32)
    weight = np.asarray(weight, dtype=np.float32)
    bias = np.asarray(bias, dtype=np.float32).reshape(-1)
    w_r = weight[int(type_index)]  # [D]

    # Host-side prep: fold the w_r row-scale into input1, lay both GEMM
    # operands out K-major, cast to fp16 (device accumulates in fp32).
    scaled = input1 * w_r[None, :]  # [N1, D]
    rhsT = np.ascontiguousarray(input2.T.astype(np.float16))  # [D, N2]
    bias_vec = np.full((P, 1), float(bias[0]), dtype=np.float32)

    in_maps = []
    for c in range(N_CORES):
        shard = scaled[c * M : (c + 1) * M]  # [M, D]
        in_maps.append(
            {
                "lhsT": np.ascontiguousarray(shard.T.astype(np.float16)),
                "rhs": rhsT,
                "biasv": bias_vec,
            }
        )

    if _cached_nc is None:
        _cached_nc = _build()

    res = run_bass_kernel_spmd(
        _cached_nc, in_maps, core_ids=list(range(N_CORES)), trace=TRACE
    )
    LAST_RESULTS = res
    return np.concatenate(
        [res.results[c]["out"] for c in range(N_CORES)], axis=0
    ).astype(np.float32)


# revision 8
# speedup vs baseline: 1.2534x; 1.0510x over previous
"""DistMult decoder kernel for 8 Trainium2 NeuronCores.

Computes out = (input1 * weight[type_index]) @ input2.T + bias with
input1 [8192, 512], input2 [8192, 512] in fp32, out [8192, 8192].

Sharding: rows of input1 (and thus rows of the output) are split across
the 8 cores; input2 / weight / bias are replicated. No communication.

Per-core device program (M = 1024 rows):
  - lhsT  [512, 1024]  = w_r-scaled shard of input1, transposed + cast
    to fp16 on host (K-major); rhs [512, 8192] = input2 transposed +
    cast to fp16 on host.  fp16 runs the PE at 1 cycle/row with fp32
    PSUM accumulation; the whole rhs (64 KB/partition) + lhsT are SBUF
    resident so there is no mid-stream prefetch.
  - compute iterates 16 column slices of 512; all 8 m-tiles accumulate
    a slice in 8 PSUM banks, so one 512 KB rhs slice unlocks ~6.9 us
    of PE work and DMA arrival order matches compute order.
  - every dma_start costs ~600 ns on its sequencer and each DGE ring
    sustains ~100-170 GB/s, so the head uses FEW, LARGE loads spread
    over all three rings in exact consumption order: sync carries
    bias + slice0, scalar carries lhsT[0:256] + slice1, gpsimd carries
    the rest of lhsT and then slices 2-15.  The PE warmup bridges the
    ~11 us until slice0 lands and the stream then runs gapless.
  - output is stored as fp16 (16 MB/core instead of 32 MB) and upcast
    to fp32 on the host.  Stores alternate sync/scalar, and from
    slice 10 the (by then idle) gpsimd ring joins the rotation so the
    store stream never backlogs into a tail drain.
  - PSUM -> SBUF copy + bias add alternates between ACT and DVE.
"""

import os

import numpy as np

import concourse.bacc as bacc
import concourse.mybir as mybir
from concourse.bass_utils import run_bass_kernel_spmd
from concourse.tile import TileContext

N_CORES = 8
N1, N2, D = 8192, 8192, 512
M = N1 // N_CORES  # rows per core
P = 128            # partitions
KT = D // P        # 4 k-tiles
MT = M // P        # 8 m-tiles
NFREE = 512        # psum bank free size (fp32)
NSL = N2 // NFREE  # 16 column slices

# test.py hooks: set TRACE=True before calling kernel() to profile; the
# BassKernelResults of the last run lands in LAST_RESULTS.
TRACE = os.environ.get("BASS_KERNEL_TRACE", "0") == "1"
LAST_RESULTS = None

_cached_nc = None


def _build():
    nc = bacc.Bacc(
        "TRN2", target_bir_lowering=False, debug=False, enable_asserts=False, num_devices=N_CORES
    )
    f32 = mybir.dt.float32
    f16 = mybir.dt.float16
    lhsT = nc.dram_tensor("lhsT", [D, M], f16, kind="ExternalInput")
    rhs = nc.dram_tensor("rhs", [D, N2], f16, kind="ExternalInput")
    biasv = nc.dram_tensor("biasv", [P, 1], f32, kind="ExternalInput")
    out = nc.dram_tensor("out", [M, N2], f16, kind="ExternalOutput")

    # K-major DRAM views split into [P, KT, cols] for single-DMA loads.
    lhsT_r = lhsT[:, :].rearrange("(kt p) m -> p kt m", p=P)
    rhs_r = rhs[:, :].rearrange("(kt p) n -> p kt n", p=P)

    with TileContext(nc) as tc:
        with (
            tc.tile_pool(name="const", bufs=1) as constp,
            tc.tile_pool(name="lhs", bufs=1) as lhsp,
            tc.tile_pool(name="rhsp", bufs=1) as rhsp,
            tc.tile_pool(name="outp", bufs=12) as outp,
            tc.tile_pool(name="psum", bufs=8, space="PSUM") as psump,
        ):
            lt = lhsp.tile([P, KT, M], f16, tag="lhs")
            rt = rhsp.tile([P, KT, N2], f16, tag="rhs")
            bias_t = constp.tile([P, 1], f32, tag="bias")

            def rt_sl(s):
                return rt[:, :, s * NFREE : (s + 1) * NFREE]

            def rhs_sl(s):
                return rhs_r[:, :, s * NFREE : (s + 1) * NFREE]

            # Warm tiles are memset by GpSimd (the earliest engine to
            # come up) so the PE warmup starts ~1 us sooner and the HAM
            # ramp to 2.4 GHz completes before the first real chain.
            warm_w = constp.tile([P, P], f16, tag="warmw")
            warm_r = constp.tile([P, NFREE], f16, tag="warmr")
            nc.gpsimd.memset(warm_w[:], 0.0)
            nc.gpsimd.memset(warm_r[:], 0.0)

            # Head: few, large loads in consumption order, with the
            # first chain's ~0.9 MB split across all three rings (each
            # ring sustains only ~100 GB/s):
            #   sync:   bias, slice0 k01, slice1 k01+k23, then stores
            #   scalar: lhsT[0:256], lhsT[256:512], then stores
            #   gpsimd: slice0 k23, lhsT[512:1024], slices 2-15
            nc.sync.dma_start(out=bias_t[:], in_=biasv[:, :])
            nc.sync.dma_start(out=rt[:, 0:2, 0:NFREE], in_=rhs_r[:, 0:2, 0:NFREE])
            nc.gpsimd.dma_start(out=rt[:, 2:4, 0:NFREE], in_=rhs_r[:, 2:4, 0:NFREE])
            nc.scalar.dma_start(out=lt[:, :, 0:256], in_=lhsT_r[:, :, 0:256])
            nc.scalar.dma_start(out=lt[:, :, 256:512], in_=lhsT_r[:, :, 256:512])
            nc.sync.dma_start(
                out=rt[:, 0:2, NFREE : 2 * NFREE],
                in_=rhs_r[:, 0:2, NFREE : 2 * NFREE],
            )
            nc.sync.dma_start(
                out=rt[:, 2:4, NFREE : 2 * NFREE],
                in_=rhs_r[:, 2:4, NFREE : 2 * NFREE],
            )
            nc.gpsimd.dma_start(out=lt[:, :, 512:768], in_=lhsT_r[:, :, 512:768])
            nc.gpsimd.dma_start(out=lt[:, :, 768:M], in_=lhsT_r[:, :, 768:M])
            for s in range(2, NSL):
                nc.gpsimd.dma_start(out=rt_sl(s), in_=rhs_sl(s))

            wps = psump.tile([P, NFREE], f32, tag="ps")
            NWARM = 14
            for i in range(NWARM):
                nc.tensor.matmul(
                    wps[:], warm_w[:], warm_r[:],
                    start=(i == 0), stop=(i == NWARM - 1),
                )

            for s in range(NSL):
                cols = slice(s * NFREE, (s + 1) * NFREE)
                for m in range(MT):
                    ps = psump.tile([P, NFREE], f32, tag="ps")
                    for k in range(KT):
                        nc.tensor.matmul(
                            ps[:], lt[:, k, m * P : (m + 1) * P],
                            rt[:, k, cols],
                            start=(k == 0), stop=(k == KT - 1),
                        )
                    ot = outp.tile([P, NFREE], f16, tag="ot")
                    # Alternate psum->sbuf+bias between ACT and the DVE so
                    # neither engine serializes the psum pool.
                    if m % 2 == 0:
                        nc.scalar.activation(
                            ot[:], ps[:],
                            mybir.ActivationFunctionType.Identity,
                            bias=bias_t[:, 0:1],
                        )
                    else:
                        nc.vector.tensor_scalar_add(ot[:], ps[:], bias_t[:, 0:1])
                    st = nc.sync if m % 2 == 0 else nc.scalar
                    st.dma_start(out=out[m * P : (m + 1) * P, cols], in_=ot[:])
    nc.compile()
    return nc


def kernel(input1, input2, weight, bias, type_index):
    global _cached_nc, LAST_RESULTS

    input1 = np.asarray(input1, dtype=np.float32)
    input2 = np.asarray(input2, dtype=np.float32)
    weight = np.asarray(weight, dtype=np.float32)
    bias = np.asarray(bias, dtype=np.float32).reshape(-1)
    w_r = weight[int(type_index)]  # [D]

    # Host-side prep: fold the w_r row-scale into input1, lay both GEMM
    # operands out K-major, cast to fp16 (device accumulates in fp32).
    scaled = input1 * w_r[None, :]  # [N1, D]
    rhsT = np.ascontiguousarray(input2.T.astype(np.float16))  # [D, N2]
    bias_vec = np.full((P, 1), float(bias[0]), dtype=np.float32)

    in_maps = []
    for c in range(N_CORES):
        shard = scaled[c * M : (c + 1) * M]  # [M, D]
        in_maps.append(
            {
                "lhsT": np.ascontiguousarray(shard.T.astype(np.float16)),
                "rhs": rhsT,
                "biasv": bias_vec,
            }
        )

    if _cached_nc is None:
        _cached_nc = _build()

    res = run_bass_kernel_spmd(
        _cached_nc, in_maps, core_ids=list(range(N_CORES)), trace=TRACE
    )
    LAST_RESULTS = res
    return np.concatenate(
        [res.results[c]["out"] for c in range(N_CORES)], axis=0
    ).astype(np.float32)
